# revision 7
# baseline (speedup 1.0000x reference)
"""ChainCRF negative-log-likelihood kernel for 8 Trainium2 NeuronCores.

Strategy
--------
The heavy part of the reference is the forward (alpha) recursion
    fv_t[b,j] = logsumexp_i(fv_{t-1}[b,i] + A[i,j]) + feat[b,t,j]
run for T steps over a 128-tag chain, batch 256.

In exp space each step is one matmul against the constant E = exp(A)
plus one elementwise multiply by ef_t = exp(feat_t) (host-prescaled so
every ef column sums to 1).  On TRN2 that step is a serial PE -> DVE
ping-pong whose latency (~535 ns: PE SBUF-access drain + DVE PSUM access
penalty + semaphore hops) cannot be reduced further, so the kernel
attacks the *number* of serial steps instead: it runs the recursion
from BOTH ends simultaneously and meets in the middle,

    forward:   q_t = ef_t * (E^T q_{t-1})          t = 1..m
    backward:  v_{t-1} = E (ef_t * v_t)            t = T-1..m+1
    partition = log(v_m . q_m)

halving the critical path.  The two chains interleave on the PE and DVE
engines (each is <60% busy) so the wall time is max(m, S) steps, not
m + S.

Sharding: data-parallel over batch.  Batch indices are sorted by length
(desc) and dealt round-robin to the 8 cores, so all cores share one
active-column profile act_t = #(slot-min lengths > t); the compiled
program shrinks the matmul free dim as sequences finish.  A column
*enters* the backward chain at its own step t = lmin_k - 1 with a
host-precomputed boundary vector (float64 backward over the per-column
leftover steps lmin_k..L_b-1, starting from E[:, END]); the entry value
ef_{lmin_k-1} * vinit is pre-folded into the initial state upload, so
variable lengths cost zero extra device instructions.

No renormalisation is needed: with colsum-1 prescaled ef the forward
state drifts only O(e^±3) over ~128 steps and the backward state is an
ef-weighted average (O(1)), both far inside bf16 range (validated in
float64/bf16 host emulation, rel err 2.4e-6).

The gold-path score is pure gather/sum over the inputs and is computed
on the host in float64.
"""

import sys

for _p in (
    "/opt/trn_rl_repo",
    "/root/.axon_site/_ro/trn_rl_repo",
    "/root/.axon_site/_ro/pypackages",
    "/root/.axon_site",
):
    if _p not in sys.path:
        sys.path.append(_p)

import numpy as np
import ml_dtypes

import concourse.bass as bass
import concourse.bacc as bacc
import concourse.tile as tile
from concourse import mybir
from concourse.bass_utils import run_bass_kernel_spmd

N_TAGS = 128
ROOT = 126
END = 127
NCORES = 8
NB = 32          # batch columns per core
CHUNK = 32       # ef DMA chunk, in time steps
CHUNK0 = 8       # first (small) chunk so compute starts early

_last_results = None      # BassKernelResults of the most recent device run
_last_nc = None           # program of the most recent device run
_last_in_maps = None      # per-core inputs of the most recent device run
_program_cache = {}       # (act profile, m, S) -> Bass program


def benchmark(n=3):
    """Re-run the last device launch n times; returns wall seconds each."""
    import time as _time

    out = []
    for _ in range(n):
        t0 = _time.time()
        run_bass_kernel_spmd(_last_nc, _last_in_maps, list(range(NCORES)))
        out.append(_time.time() - t0)
    return out


def _chunk_bounds(nslices):
    """[(start, end)] DMA chunks over a stream of nslices step-slices."""
    bounds = [(0, min(CHUNK0, nslices))]
    t = CHUNK0
    while t < nslices:
        bounds.append((t, min(t + CHUNK, nslices)))
        t += CHUNK
    return bounds


def _build_program(act, m, S, Tdev):
    """One SPMD program shared by all 8 cores.

    act[t] = number of active batch columns at step t (non-increasing).
    Forward chain: steps t = 1..m.  Backward chain: steps s = 0..S-1
    (s maps to t = Tdev-1-s).  S = Tdev - 1 - m.
    """
    f32 = mybir.dt.float32
    bf16 = mybir.dt.bfloat16
    a_last = act[m + 1]

    nc = bacc.Bacc("TRN2", debug=False, num_devices=NCORES)
    e_d = nc.dram_tensor("emat", [N_TAGS, N_TAGS], bf16, kind="ExternalInput")
    et_d = nc.dram_tensor("ematT", [N_TAGS, N_TAGS], bf16, kind="ExternalInput")
    eff_d = nc.dram_tensor("eff", [N_TAGS, (m + 1) * NB], bf16, kind="ExternalInput")
    efb_d = nc.dram_tensor("efb", [N_TAGS, S * NB], bf16, kind="ExternalInput")
    u0_d = nc.dram_tensor("u0", [N_TAGS, NB], bf16, kind="ExternalInput")
    qout_d = nc.dram_tensor("q_out", [N_TAGS, NB], bf16, kind="ExternalOutput")
    vout_d = nc.dram_tensor("v_out", [N_TAGS, NB], f32, kind="ExternalOutput")

    fbounds = _chunk_bounds(m + 1)
    bbounds = _chunk_bounds(S)

    with tile.TileContext(nc) as tc:
        with (
            tc.tile_pool(name="const", bufs=1) as const_pool,
            tc.tile_pool(name="effp", bufs=1) as eff_pool,
            tc.tile_pool(name="efbp", bufs=1) as efb_pool,
            tc.tile_pool(name="state", bufs=1) as state_pool,
            tc.tile_pool(name="pf", bufs=2, space="PSUM") as pf_pool,
            tc.tile_pool(name="pb", bufs=2, space="PSUM") as pb_pool,
        ):
            # first chunks + constants first so compute starts early
            eff_tiles = [None] * len(fbounds)
            efb_tiles = [None] * len(bbounds)

            def dma_chunk(which, ci):
                t0, t1 = (fbounds if which == "f" else bbounds)[ci]
                pool = eff_pool if which == "f" else efb_pool
                src = eff_d if which == "f" else efb_d
                et = pool.tile([N_TAGS, (t1 - t0) * NB], bf16, tag=f"e{which}{t0}")
                nc.sync.dma_start(et[:], src[:, t0 * NB : t1 * NB])
                (eff_tiles if which == "f" else efb_tiles)[ci] = et

            # constants on the Activation HWDGE queue, streams on SP: the
            # first matmul's inputs arrive via two parallel DMA paths
            e_t = const_pool.tile([N_TAGS, N_TAGS], bf16, tag="emat")
            nc.scalar.dma_start(e_t[:], e_d[:])
            dma_chunk("f", 0)
            et_t = const_pool.tile([N_TAGS, N_TAGS], bf16, tag="ematT")
            nc.scalar.dma_start(et_t[:], et_d[:])
            u = state_pool.tile([N_TAGS, NB], bf16, tag="u")
            nc.scalar.dma_start(u[:], u0_d[:])
            dma_chunk("b", 0)
            for ci in range(1, max(len(fbounds), len(bbounds))):
                if ci < len(fbounds):
                    dma_chunk("f", ci)
                if ci < len(bbounds):
                    dma_chunk("b", ci)

            def eslice(which, t, width):
                bounds = fbounds if which == "f" else bbounds
                tiles = eff_tiles if which == "f" else efb_tiles
                for (t0, t1), et in zip(bounds, tiles):
                    if t0 <= t < t1:
                        return et[:, (t - t0) * NB : (t - t0) * NB + width]
                raise AssertionError(t)

            q = state_pool.tile([N_TAGS, NB], bf16, tag="q")
            nc.vector.tensor_copy(q[:], eslice("f", 0, NB))

            wprev = None
            for i in range(1, max(m, S) + 1):
                t = i           # forward step
                s = i - 1       # backward step
                af = act[t] if t <= m else 0
                mmf = None
                if af > 0:
                    mmf = pf_pool.tile([N_TAGS, NB], f32, tag="pf")
                    nc.tensor.matmul(
                        mmf[:, :af], e_t[:, :], q[:, :af], start=True, stop=True
                    )
                if s <= S - 1:
                    tb = Tdev - 1 - s
                    ab = act[tb]
                    abprev = act[tb + 1] if s >= 1 else 0
                    if abprev > 0:
                        nc.vector.tensor_mul(
                            u[:, :abprev], wprev[:, :abprev],
                            eslice("b", s, abprev),
                        )
                    if ab > 0:
                        mmb = pb_pool.tile([N_TAGS, NB], f32, tag="pb")
                        nc.tensor.matmul(
                            mmb[:, :ab], et_t[:, :], u[:, :ab],
                            start=True, stop=True,
                        )
                        wprev = mmb
                if mmf is not None:
                    nc.vector.tensor_mul(
                        q[:, :af], mmf[:, :af], eslice("f", t, af)
                    )

            nc.sync.dma_start(qout_d[:], q[:])
            vsb = state_pool.tile([N_TAGS, NB], f32, tag="vsb")
            nc.vector.tensor_copy(vsb[:, :a_last], wprev[:, :a_last])
            nc.scalar.dma_start(vout_d[:, :a_last], vsb[:, :a_last])

    nc.finalize()
    return nc


def kernel(feats, tags, mask, log_transitions):
    global _last_results, _last_nc, _last_in_maps
    feats = np.asarray(feats, dtype=np.float32)
    tags = np.asarray(tags)
    mask = np.asarray(mask)
    lt = np.asarray(log_transitions, dtype=np.float32)
    bsz, T, n = feats.shape
    assert (bsz, T, n) == (256, 256, N_TAGS)

    lengths = mask.astype(np.int64).sum(1)
    order = np.argsort(-lengths, kind="stable")  # desc
    lmin = lengths[order[7::8]]                  # slot-min profile, len NB
    Tdev = int(lmin[0])
    m = (Tdev - 1) // 2
    S = Tdev - 1 - m
    act = [int((lmin > t).sum()) for t in range(Tdev + 2)]

    E64 = np.exp(lt.astype(np.float64))
    Ebf = E64.astype(np.float32).astype(ml_dtypes.bfloat16)
    EtBf = np.ascontiguousarray(E64.T).astype(np.float32).astype(ml_dtypes.bfloat16)
    w64 = E64[:, END]

    feats64 = feats.astype(np.float64)
    lt64 = lt.astype(np.float64)

    in_maps = []
    corr_all = np.zeros((NCORES, NB))
    vlog_all = np.zeros((NCORES, NB))
    idx_all = np.zeros((NCORES, NB), np.int64)
    vinit_all = np.zeros((NCORES, N_TAGS, NB))
    for c in range(NCORES):
        idx = order[c::8][:NB]
        idx_all[c] = idx
        f = feats64[idx, :Tdev, :]               # [NB, Tdev, 128]
        ef = np.exp(f)
        ef[:, 0, :] *= np.exp(lt64[ROOT])[None, :]
        s = ef.sum(axis=2)                       # [NB, Tdev]
        ef /= s[:, :, None]
        tgrid = np.arange(Tdev)[None, :]
        corr_all[c] = (np.log(s) * (tgrid < lmin[:, None])).sum(axis=1)

        # boundary vector per column: float64 backward over the leftover
        # steps L_b-1..lmin_k (exact), starting from w = E[:, END]
        vinit = np.zeros((N_TAGS, NB))
        for k in range(NB):
            b = idx[k]
            v = w64.copy()
            for t in range(int(lengths[b]) - 1, int(lmin[k]) - 1, -1):
                v = E64 @ (np.exp(feats64[b, t]) * v)
                sc = v.sum()
                v /= sc
                vlog_all[c, k] += np.log(sc)
            vinit[:, k] = v
        vinit_all[c] = vinit

        # streams: eff slice t = ef_t (t = 0..m);  efb slice s = ef_{Tdev-1-s}
        eff = np.ascontiguousarray(
            ef[:, : m + 1, :].transpose(2, 1, 0), dtype=np.float32
        ).reshape(N_TAGS, (m + 1) * NB).astype(ml_dtypes.bfloat16)
        efb = np.ascontiguousarray(
            ef[:, :m:-1, :].transpose(2, 1, 0), dtype=np.float32
        ).reshape(N_TAGS, S * NB).astype(ml_dtypes.bfloat16)

        # initial backward state: entry value pre-folded for columns that
        # enter the device backward chain; plain vinit otherwise
        u0 = vinit.copy()
        for k in range(NB):
            if int(lmin[k]) > m + 1:
                u0[:, k] = ef[k, int(lmin[k]) - 1, :] * vinit[:, k]
        u0 = u0.astype(np.float32).astype(ml_dtypes.bfloat16)

        in_maps.append(
            {"emat": Ebf, "ematT": EtBf, "eff": eff, "efb": efb, "u0": u0}
        )

    key = (tuple(act), m, S)
    if key not in _program_cache:
        _program_cache[key] = _build_program(act, m, S, Tdev)
    nc = _program_cache[key]

    _last_nc, _last_in_maps = nc, in_maps
    res = run_bass_kernel_spmd(nc, in_maps, list(range(NCORES)))
    _last_results = res

    # --- host assembly (float64) ---
    partition = np.zeros(bsz)
    for c in range(NCORES):
        qf = res.results[c]["q_out"].astype(np.float64)          # [128, NB]
        vf = res.results[c]["v_out"].astype(np.float64)          # [128, NB]
        for k in range(NB):
            b = idx_all[c, k]
            vk = vf[:, k] if int(lmin[k]) > m + 1 else vinit_all[c][:, k]
            partition[b] = (
                np.log(vk @ qf[:, k]) + corr_all[c, k] + vlog_all[c, k]
            )

    # --- gold path score (host, float64) ---
    maskf = mask.astype(np.float64)
    trans_tt = lt64[tags[:, :-1], tags[:, 1:]]
    emis = np.take_along_axis(
        feats64[:, :-1, :], tags[:, :-1, None].astype(np.int64), axis=2
    )[..., 0]
    scores = lt64[ROOT, tags[:, 0]]
    scores = scores + (trans_tt * maskf[:, 1:] + emis * maskf[:, :-1]).sum(axis=1)
    last_idx = (maskf.sum(axis=1) - 1.0).astype(np.int64)
    last_tags = np.take_along_axis(np.asarray(tags, np.int64), last_idx[:, None], axis=1)[:, 0]
    last_input = np.take_along_axis(feats64[:, -1, :], last_tags[:, None], axis=1)[:, 0]
    scores = scores + lt64[last_tags, END] + last_input * maskf[:, -1]

    return np.asarray((partition - scores).mean(), dtype=np.float32)


# revision 9
# speedup vs baseline: 1.0251x; 1.0251x over previous
"""ChainCRF negative-log-likelihood kernel for 8 Trainium2 NeuronCores.

Strategy
--------
The heavy part of the reference is the forward (alpha) recursion
    fv_t[b,j] = logsumexp_i(fv_{t-1}[b,i] + A[i,j]) + feat[b,t,j]
run for T steps over a 128-tag chain, batch 256.

In exp space each step is one matmul against the constant E = exp(A)
plus one elementwise multiply by ef_t = exp(feat_t) (host-prescaled so
every ef column sums to 1).  On TRN2 that step is a serial PE -> DVE
ping-pong whose latency (~535 ns: PE SBUF-access drain + DVE PSUM access
penalty + semaphore hops) cannot be reduced further, so the kernel
attacks the *number* of serial steps instead: it runs the recursion
from BOTH ends simultaneously and meets in the middle,

    forward:   q_t = ef_t * (E^T q_{t-1})          t = 1..m
    backward:  v_{t-1} = E (ef_t * v_t)            t = T-1..m+1
    partition = log(v_m . q_m)

halving the critical path.  The two chains interleave on the PE and DVE
engines (each is <60% busy) so the wall time is max(m, S) steps, not
m + S.

Sharding: data-parallel over batch.  Batch indices are sorted by length
(desc) and dealt round-robin to the 8 cores, so all cores share one
active-column profile act_t = #(slot-min lengths > t); the compiled
program shrinks the matmul free dim as sequences finish.  A column
*enters* the backward chain at its own step t = lmin_k - 1 with a
host-precomputed boundary vector (float64 backward over the per-column
leftover steps lmin_k..L_b-1, starting from E[:, END]); the entry value
ef_{lmin_k-1} * vinit is pre-folded into the initial state upload, so
variable lengths cost zero extra device instructions.

No renormalisation is needed: with colsum-1 prescaled ef the forward
state drifts only O(e^±3) over ~128 steps and the backward state is an
ef-weighted average (O(1)), both far inside bf16 range (validated in
float64/bf16 host emulation, rel err 2.4e-6).

The gold-path score is pure gather/sum over the inputs and is computed
on the host in float64.
"""

import sys

for _p in (
    "/opt/trn_rl_repo",
    "/root/.axon_site/_ro/trn_rl_repo",
    "/root/.axon_site/_ro/pypackages",
    "/root/.axon_site",
):
    if _p not in sys.path:
        sys.path.append(_p)

import numpy as np
import ml_dtypes

import concourse.bass as bass
import concourse.bacc as bacc
import concourse.tile as tile
from concourse import mybir
from concourse.bass_utils import run_bass_kernel_spmd

N_TAGS = 128
ROOT = 126
END = 127
NCORES = 8
NB = 32          # batch columns per core
CHUNK = 32       # ef DMA chunk, in time steps
CHUNK0 = 8       # first (small) chunk so compute starts early

_last_results = None      # BassKernelResults of the most recent device run
_last_nc = None           # program of the most recent device run
_last_in_maps = None      # per-core inputs of the most recent device run
_program_cache = {}       # (act profile, m, S) -> Bass program


def benchmark(n=3):
    """Re-run the last device launch n times; returns wall seconds each."""
    import time as _time

    out = []
    for _ in range(n):
        t0 = _time.time()
        run_bass_kernel_spmd(_last_nc, _last_in_maps, list(range(NCORES)))
        out.append(_time.time() - t0)
    return out


def _chunk_bounds(nslices):
    """[(start, end)] DMA chunks over a stream of nslices step-slices."""
    bounds = [(0, min(CHUNK0, nslices))]
    t = CHUNK0
    while t < nslices:
        bounds.append((t, min(t + CHUNK, nslices)))
        t += CHUNK
    return bounds


def _build_program(act, m, S, Tdev):
    """One SPMD program shared by all 8 cores.

    act[t] = number of active batch columns at step t (non-increasing).
    Forward chain: steps t = 1..m.  Backward chain: steps s = 0..S-1
    (s maps to t = Tdev-1-s).  S = Tdev - 1 - m.
    """
    f32 = mybir.dt.float32
    bf16 = mybir.dt.bfloat16
    a_last = act[m + 1]

    # boot tensor: everything the first CHUNK0 iterations of both chains
    # need, in ONE DMA (HWDGE descriptor-gen is a serialized shared
    # resource; five separate DMAs would delay steady state to ~5 us)
    c0f = min(CHUNK0, m + 1)
    c0b = min(CHUNK0, S)
    OFF_ET = N_TAGS
    OFF_U = 2 * N_TAGS
    OFF_EFF = 2 * N_TAGS + NB
    OFF_EFB = OFF_EFF + c0f * NB
    BOOT_W = OFF_EFB + c0b * NB

    nc = bacc.Bacc("TRN2", debug=False, num_devices=NCORES)
    boot_d = nc.dram_tensor("boot", [N_TAGS, BOOT_W], bf16, kind="ExternalInput")
    eff_d = nc.dram_tensor("eff", [N_TAGS, (m + 1) * NB], bf16, kind="ExternalInput")
    efb_d = nc.dram_tensor("efb", [N_TAGS, S * NB], bf16, kind="ExternalInput")
    qout_d = nc.dram_tensor("q_out", [N_TAGS, NB], bf16, kind="ExternalOutput")
    vout_d = nc.dram_tensor("v_out", [N_TAGS, NB], f32, kind="ExternalOutput")

    fbounds = [(0, c0f)] + _chunk_bounds(m + 1)[1:]
    bbounds = [(0, c0b)] + _chunk_bounds(S)[1:]

    with tile.TileContext(nc) as tc:
        with (
            tc.tile_pool(name="boot", bufs=1) as boot_pool,
            tc.tile_pool(name="effp", bufs=1) as eff_pool,
            tc.tile_pool(name="efbp", bufs=1) as efb_pool,
            tc.tile_pool(name="state", bufs=1) as state_pool,
            tc.tile_pool(name="pf", bufs=2, space="PSUM") as pf_pool,
            tc.tile_pool(name="pb", bufs=2, space="PSUM") as pb_pool,
        ):
            boot = boot_pool.tile([N_TAGS, BOOT_W], bf16, tag="boot")
            nc.sync.dma_start(boot[:], boot_d[:])
            e_t = boot[:, 0:N_TAGS]
            et_t = boot[:, OFF_ET : OFF_ET + N_TAGS]
            u = boot[:, OFF_U : OFF_U + NB]

            eff_tiles = [boot] * len(fbounds)
            efb_tiles = [boot] * len(bbounds)

            def dma_chunk(which, ci):
                t0, t1 = (fbounds if which == "f" else bbounds)[ci]
                pool = eff_pool if which == "f" else efb_pool
                src = eff_d if which == "f" else efb_d
                et = pool.tile([N_TAGS, (t1 - t0) * NB], bf16, tag=f"e{which}{t0}")
                nc.sync.dma_start(et[:], src[:, t0 * NB : t1 * NB])
                (eff_tiles if which == "f" else efb_tiles)[ci] = et

            for ci in range(1, max(len(fbounds), len(bbounds))):
                if ci < len(fbounds):
                    dma_chunk("f", ci)
                if ci < len(bbounds):
                    dma_chunk("b", ci)

            def eslice(which, t, width):
                bounds = fbounds if which == "f" else bbounds
                tiles = eff_tiles if which == "f" else efb_tiles
                boot_off = OFF_EFF if which == "f" else OFF_EFB
                for ci, ((t0, t1), et) in enumerate(zip(bounds, tiles)):
                    if t0 <= t < t1:
                        base = boot_off if ci == 0 else 0
                        lo = base + (t - t0) * NB
                        return et[:, lo : lo + width]
                raise AssertionError(t)

            q = state_pool.tile([N_TAGS, NB], bf16, tag="q")
            nc.vector.memset(q[:], 0.0)

            wprev = None
            for i in range(1, max(m, S) + 1):
                t = i           # forward step
                s = i - 1       # backward step
                af = act[t] if t <= m else 0
                mmf = None
                if af > 0:
                    rhs = eslice("f", 0, af) if t == 1 else q[:, :af]
                    mmf = pf_pool.tile([N_TAGS, NB], f32, tag="pf")
                    nc.tensor.matmul(
                        mmf[:, :af], e_t[:, :], rhs, start=True, stop=True
                    )
                if s <= S - 1:
                    tb = Tdev - 1 - s
                    ab = act[tb]
                    abprev = act[tb + 1] if s >= 1 else 0
                    if abprev > 0:
                        nc.vector.tensor_mul(
                            u[:, :abprev], wprev[:, :abprev],
                            eslice("b", s, abprev),
                        )
                    if ab > 0:
                        mmb = pb_pool.tile([N_TAGS, NB], f32, tag="pb")
                        nc.tensor.matmul(
                            mmb[:, :ab], et_t[:, :], u[:, :ab],
                            start=True, stop=True,
                        )
                        wprev = mmb
                if mmf is not None:
                    nc.vector.tensor_mul(
                        q[:, :af], mmf[:, :af], eslice("f", t, af)
                    )

            nc.scalar.dma_start(qout_d[:], q[:])
            vsb = state_pool.tile([N_TAGS, NB], f32, tag="vsb")
            nc.vector.tensor_copy(vsb[:, :a_last], wprev[:, :a_last])
            nc.sync.dma_start(vout_d[:, :a_last], vsb[:, :a_last])

    nc.finalize()
    return nc


def kernel(feats, tags, mask, log_transitions):
    global _last_results, _last_nc, _last_in_maps
    feats = np.asarray(feats, dtype=np.float32)
    tags = np.asarray(tags)
    mask = np.asarray(mask)
    lt = np.asarray(log_transitions, dtype=np.float32)
    bsz, T, n = feats.shape
    assert (bsz, T, n) == (256, 256, N_TAGS)

    lengths = mask.astype(np.int64).sum(1)
    order = np.argsort(-lengths, kind="stable")  # desc
    lmin = lengths[order[7::8]]                  # slot-min profile, len NB
    Tdev = int(lmin[0])
    m = (Tdev - 1) // 2
    S = Tdev - 1 - m
    act = [int((lmin > t).sum()) for t in range(Tdev + 2)]

    E64 = np.exp(lt.astype(np.float64))
    Ebf = E64.astype(np.float32).astype(ml_dtypes.bfloat16)
    EtBf = np.ascontiguousarray(E64.T).astype(np.float32).astype(ml_dtypes.bfloat16)
    w64 = E64[:, END]

    feats64 = feats.astype(np.float64)
    lt64 = lt.astype(np.float64)

    c0f = min(CHUNK0, m + 1)
    c0b = min(CHUNK0, S)
    in_maps = []
    corr_all = np.zeros((NCORES, NB))
    vlog_all = np.zeros((NCORES, NB))
    idx_all = np.zeros((NCORES, NB), np.int64)
    vinit_all = np.zeros((NCORES, N_TAGS, NB))
    ef0_all = np.zeros((NCORES, N_TAGS, NB))
    for c in range(NCORES):
        idx = order[c::8][:NB]
        idx_all[c] = idx
        f = feats64[idx, :Tdev, :]               # [NB, Tdev, 128]
        ef = np.exp(f)
        ef[:, 0, :] *= np.exp(lt64[ROOT])[None, :]
        s = ef.sum(axis=2)                       # [NB, Tdev]
        ef /= s[:, :, None]
        tgrid = np.arange(Tdev)[None, :]
        corr_all[c] = (np.log(s) * (tgrid < lmin[:, None])).sum(axis=1)
        ef0_all[c] = ef[:, 0, :].T

        # boundary vector per column: float64 backward over the leftover
        # steps L_b-1..lmin_k (exact), starting from w = E[:, END]
        vinit = np.zeros((N_TAGS, NB))
        for k in range(NB):
            b = idx[k]
            v = w64.copy()
            for t in range(int(lengths[b]) - 1, int(lmin[k]) - 1, -1):
                v = E64 @ (np.exp(feats64[b, t]) * v)
                sc = v.sum()
                v /= sc
                vlog_all[c, k] += np.log(sc)
            vinit[:, k] = v
        vinit_all[c] = vinit

        # streams: eff slice t = ef_t (t = 0..m);  efb slice s = ef_{Tdev-1-s}
        eff = np.ascontiguousarray(
            ef[:, : m + 1, :].transpose(2, 1, 0), dtype=np.float32
        ).reshape(N_TAGS, (m + 1) * NB).astype(ml_dtypes.bfloat16)
        efb = np.ascontiguousarray(
            ef[:, :m:-1, :].transpose(2, 1, 0), dtype=np.float32
        ).reshape(N_TAGS, S * NB).astype(ml_dtypes.bfloat16)

        # initial backward state: entry value pre-folded for columns that
        # enter the device backward chain; plain vinit otherwise
        u0 = vinit.copy()
        for k in range(NB):
            if int(lmin[k]) > m + 1:
                u0[:, k] = ef[k, int(lmin[k]) - 1, :] * vinit[:, k]
        u0 = u0.astype(np.float32).astype(ml_dtypes.bfloat16)

        boot = np.concatenate(
            [Ebf, EtBf, u0, eff[:, : c0f * NB], efb[:, : c0b * NB]], axis=1
        )
        in_maps.append({"boot": boot, "eff": eff, "efb": efb})

    key = (tuple(act), m, S)
    if key not in _program_cache:
        _program_cache[key] = _build_program(act, m, S, Tdev)
    nc = _program_cache[key]

    _last_nc, _last_in_maps = nc, in_maps
    res = run_bass_kernel_spmd(nc, in_maps, list(range(NCORES)))
    _last_results = res

    # --- host assembly (float64) ---
    partition = np.zeros(bsz)
    for c in range(NCORES):
        qf = res.results[c]["q_out"].astype(np.float64)          # [128, NB]
        vf = res.results[c]["v_out"].astype(np.float64)          # [128, NB]
        for k in range(NB):
            b = idx_all[c, k]
            vk = vf[:, k] if int(lmin[k]) > m + 1 else vinit_all[c][:, k]
            # columns with lmin == 1 never enter the device forward chain
            qk = qf[:, k] if int(lmin[k]) > 1 else ef0_all[c][:, k]
            partition[b] = (
                np.log(vk @ qk) + corr_all[c, k] + vlog_all[c, k]
            )

    # --- gold path score (host, float64) ---
    maskf = mask.astype(np.float64)
    trans_tt = lt64[tags[:, :-1], tags[:, 1:]]
    emis = np.take_along_axis(
        feats64[:, :-1, :], tags[:, :-1, None].astype(np.int64), axis=2
    )[..., 0]
    scores = lt64[ROOT, tags[:, 0]]
    scores = scores + (trans_tt * maskf[:, 1:] + emis * maskf[:, :-1]).sum(axis=1)
    last_idx = (maskf.sum(axis=1) - 1.0).astype(np.int64)
    last_tags = np.take_along_axis(np.asarray(tags, np.int64), last_idx[:, None], axis=1)[:, 0]
    last_input = np.take_along_axis(feats64[:, -1, :], last_tags[:, None], axis=1)[:, 0]
    scores = scores + lt64[last_tags, END] + last_input * maskf[:, -1]

    return np.asarray((partition - scores).mean(), dtype=np.float32)


# revision 11
# speedup vs baseline: 1.0352x; 1.0099x over previous
"""ChainCRF negative-log-likelihood kernel for 8 Trainium2 NeuronCores.

Strategy
--------
The heavy part of the reference is the forward (alpha) recursion
    fv_t[b,j] = logsumexp_i(fv_{t-1}[b,i] + A[i,j]) + feat[b,t,j]
run for T steps over a 128-tag chain, batch 256.

In exp space each step is one matmul against the constant E = exp(A)
plus one elementwise multiply by ef_t = exp(feat_t) (host-prescaled so
every ef column sums to 1).  On TRN2 that step is a serial PE -> DVE
ping-pong whose latency (~535 ns: PE SBUF-access drain + DVE PSUM access
penalty + semaphore hops) cannot be reduced further, so the kernel
attacks the *number* of serial steps instead: it runs the recursion
from BOTH ends simultaneously and meets in the middle,

    forward:   q_t = ef_t * (E^T q_{t-1})          t = 1..m
    backward:  v_{t-1} = E (ef_t * v_t)            t = T-1..m+1
    partition = log(v_m . q_m)

halving the critical path.  The two chains interleave on the PE and DVE
engines (each is <60% busy) so the wall time is max(m, S) steps, not
m + S.

Sharding: data-parallel over batch.  Batch indices are sorted by length
(desc) and dealt round-robin to the 8 cores, so all cores share one
active-column profile act_t = #(slot-min lengths > t); the compiled
program shrinks the matmul free dim as sequences finish.  A column
*enters* the backward chain at its own step t = lmin_k - 1 with a
host-precomputed boundary vector (float64 backward over the per-column
leftover steps lmin_k..L_b-1, starting from E[:, END]); the entry value
ef_{lmin_k-1} * vinit is pre-folded into the initial state upload, so
variable lengths cost zero extra device instructions.

No renormalisation is needed: with colsum-1 prescaled ef the forward
state drifts only O(e^±3) over ~128 steps and the backward state is an
ef-weighted average (O(1)), both far inside bf16 range (validated in
float64/bf16 host emulation, rel err 2.4e-6).

The gold-path score is pure gather/sum over the inputs and is computed
on the host in float64.
"""

import sys

for _p in (
    "/opt/trn_rl_repo",
    "/root/.axon_site/_ro/trn_rl_repo",
    "/root/.axon_site/_ro/pypackages",
    "/root/.axon_site",
):
    if _p not in sys.path:
        sys.path.append(_p)

import numpy as np
import ml_dtypes

import concourse.bass as bass
import concourse.bacc as bacc
import concourse.tile as tile
from concourse import mybir
from concourse.bass_utils import run_bass_kernel_spmd

N_TAGS = 128
ROOT = 126
END = 127
NCORES = 8
NB = 32          # batch columns per core
CHUNK = 32       # ef DMA chunk, in time steps
CHUNK0 = 4       # boot chunk, in time steps (rides the boot DMA)

_last_results = None      # BassKernelResults of the most recent device run
_last_nc = None           # program of the most recent device run
_last_in_maps = None      # per-core inputs of the most recent device run
_program_cache = {}       # (act profile, m, S) -> Bass program


def benchmark(n=3):
    """Re-run the last device launch n times; returns wall seconds each."""
    import time as _time

    out = []
    for _ in range(n):
        t0 = _time.time()
        run_bass_kernel_spmd(_last_nc, _last_in_maps, list(range(NCORES)))
        out.append(_time.time() - t0)
    return out


def _chunk_bounds(nslices):
    """[(start, end)] DMA chunks over a stream of nslices step-slices."""
    bounds = [(0, min(CHUNK0, nslices))]
    t = CHUNK0
    while t < nslices:
        bounds.append((t, min(t + CHUNK, nslices)))
        t += CHUNK
    return bounds


def _build_program(act, m, S, Tdev):
    """One SPMD program shared by all 8 cores.

    act[t] = number of active batch columns at step t (non-increasing).
    Forward chain: steps t = 1..m.  Backward chain: steps s = 0..S-1
    (s maps to t = Tdev-1-s).  S = Tdev - 1 - m.
    """
    f32 = mybir.dt.float32
    bf16 = mybir.dt.bfloat16
    a_last = act[m + 1]

    # boot tensor: everything the first CHUNK0 iterations of both chains
    # need, in ONE DMA (HWDGE descriptor-gen is a serialized shared
    # resource; five separate DMAs would delay steady state to ~5 us)
    c0f = min(CHUNK0, m + 1)
    c0b = min(CHUNK0, S)
    OFF_ET = N_TAGS
    OFF_U = 2 * N_TAGS
    OFF_EFF = 2 * N_TAGS + NB
    OFF_EFB = OFF_EFF + c0f * NB
    BOOT_W = OFF_EFB + c0b * NB

    nc = bacc.Bacc("TRN2", debug=False, num_devices=NCORES)
    boot_d = nc.dram_tensor("boot", [N_TAGS, BOOT_W], bf16, kind="ExternalInput")
    eff_d = nc.dram_tensor("eff", [N_TAGS, (m + 1) * NB], bf16, kind="ExternalInput")
    efb_d = nc.dram_tensor("efb", [N_TAGS, S * NB], bf16, kind="ExternalInput")
    out_d = nc.dram_tensor("qv_out", [N_TAGS, 2 * NB], bf16, kind="ExternalOutput")

    fbounds = [(0, c0f)] + _chunk_bounds(m + 1)[1:]
    bbounds = [(0, c0b)] + _chunk_bounds(S)[1:]

    with tile.TileContext(nc) as tc:
        with (
            tc.tile_pool(name="boot", bufs=1) as boot_pool,
            tc.tile_pool(name="effp", bufs=1) as eff_pool,
            tc.tile_pool(name="efbp", bufs=1) as efb_pool,
            tc.tile_pool(name="state", bufs=1) as state_pool,
            tc.tile_pool(name="pf", bufs=2, space="PSUM") as pf_pool,
            tc.tile_pool(name="pb", bufs=2, space="PSUM") as pb_pool,
        ):
            boot = boot_pool.tile([N_TAGS, BOOT_W], bf16, tag="boot")
            nc.sync.dma_start(boot[:], boot_d[:])
            e_t = boot[:, 0:N_TAGS]
            et_t = boot[:, OFF_ET : OFF_ET + N_TAGS]
            u = boot[:, OFF_U : OFF_U + NB]

            eff_tiles = [boot] * len(fbounds)
            efb_tiles = [boot] * len(bbounds)

            def dma_chunk(which, ci):
                t0, t1 = (fbounds if which == "f" else bbounds)[ci]
                pool = eff_pool if which == "f" else efb_pool
                src = eff_d if which == "f" else efb_d
                et = pool.tile([N_TAGS, (t1 - t0) * NB], bf16, tag=f"e{which}{t0}")
                nc.sync.dma_start(et[:], src[:, t0 * NB : t1 * NB])
                (eff_tiles if which == "f" else efb_tiles)[ci] = et

            for ci in range(1, max(len(fbounds), len(bbounds))):
                if ci < len(fbounds):
                    dma_chunk("f", ci)
                if ci < len(bbounds):
                    dma_chunk("b", ci)

            def eslice(which, t, width):
                bounds = fbounds if which == "f" else bbounds
                tiles = eff_tiles if which == "f" else efb_tiles
                boot_off = OFF_EFF if which == "f" else OFF_EFB
                for ci, ((t0, t1), et) in enumerate(zip(bounds, tiles)):
                    if t0 <= t < t1:
                        base = boot_off if ci == 0 else 0
                        lo = base + (t - t0) * NB
                        return et[:, lo : lo + width]
                raise AssertionError(t)

            qv = state_pool.tile([N_TAGS, 2 * NB], bf16, tag="qv")
            nc.vector.memset(qv[:], 0.0)
            q = qv  # forward state lives in columns [0, NB); af <= NB always

            wprev = None
            for i in range(1, max(m, S) + 1):
                t = i           # forward step
                s = i - 1       # backward step
                af = act[t] if t <= m else 0
                mmf = None
                if af > 0:
                    rhs = eslice("f", 0, af) if t == 1 else q[:, :af]
                    mmf = pf_pool.tile([N_TAGS, NB], f32, tag="pf")
                    nc.tensor.matmul(
                        mmf[:, :af], e_t[:, :], rhs, start=True, stop=True
                    )
                if s <= S - 1:
                    tb = Tdev - 1 - s
                    ab = act[tb]
                    abprev = act[tb + 1] if s >= 1 else 0
                    if abprev > 0:
                        nc.vector.tensor_mul(
                            u[:, :abprev], wprev[:, :abprev],
                            eslice("b", s, abprev),
                        )
                    if ab > 0:
                        mmb = pb_pool.tile([N_TAGS, NB], f32, tag="pb")
                        nc.tensor.matmul(
                            mmb[:, :ab], et_t[:, :], u[:, :ab],
                            start=True, stop=True,
                        )
                        wprev = mmb
                if mmf is not None:
                    nc.vector.tensor_mul(
                        q[:, :af], mmf[:, :af], eslice("f", t, af)
                    )

            nc.vector.tensor_copy(qv[:, NB : NB + a_last], wprev[:, :a_last])
            nc.sync.dma_start(out_d[:, : NB + a_last], qv[:, : NB + a_last])

    nc.finalize()
    return nc


def kernel(feats, tags, mask, log_transitions):
    global _last_results, _last_nc, _last_in_maps
    feats = np.asarray(feats, dtype=np.float32)
    tags = np.asarray(tags)
    mask = np.asarray(mask)
    lt = np.asarray(log_transitions, dtype=np.float32)
    bsz, T, n = feats.shape
    assert (bsz, T, n) == (256, 256, N_TAGS)

    lengths = mask.astype(np.int64).sum(1)
    order = np.argsort(-lengths, kind="stable")  # desc
    lmin = lengths[order[7::8]]                  # slot-min profile, len NB
    Tdev = int(lmin[0])
    m = (Tdev - 1) // 2
    S = Tdev - 1 - m
    act = [int((lmin > t).sum()) for t in range(Tdev + 2)]

    E64 = np.exp(lt.astype(np.float64))
    Ebf = E64.astype(np.float32).astype(ml_dtypes.bfloat16)
    EtBf = np.ascontiguousarray(E64.T).astype(np.float32).astype(ml_dtypes.bfloat16)
    w64 = E64[:, END]

    feats64 = feats.astype(np.float64)
    lt64 = lt.astype(np.float64)

    c0f = min(CHUNK0, m + 1)
    c0b = min(CHUNK0, S)
    in_maps = []
    corr_all = np.zeros((NCORES, NB))
    vlog_all = np.zeros((NCORES, NB))
    idx_all = np.zeros((NCORES, NB), np.int64)
    vinit_all = np.zeros((NCORES, N_TAGS, NB))
    ef0_all = np.zeros((NCORES, N_TAGS, NB))
    for c in range(NCORES):
        idx = order[c::8][:NB]
        idx_all[c] = idx
        f = feats64[idx, :Tdev, :]               # [NB, Tdev, 128]
        ef = np.exp(f)
        ef[:, 0, :] *= np.exp(lt64[ROOT])[None, :]
        s = ef.sum(axis=2)                       # [NB, Tdev]
        ef /= s[:, :, None]
        tgrid = np.arange(Tdev)[None, :]
        corr_all[c] = (np.log(s) * (tgrid < lmin[:, None])).sum(axis=1)
        ef0_all[c] = ef[:, 0, :].T

        # boundary vector per column: float64 backward over the leftover
        # steps L_b-1..lmin_k (exact), starting from w = E[:, END]
        vinit = np.zeros((N_TAGS, NB))
        for k in range(NB):
            b = idx[k]
            v = w64.copy()
            for t in range(int(lengths[b]) - 1, int(lmin[k]) - 1, -1):
                v = E64 @ (np.exp(feats64[b, t]) * v)
                sc = v.sum()
                v /= sc
                vlog_all[c, k] += np.log(sc)
            vinit[:, k] = v
        vinit_all[c] = vinit

        # streams: eff slice t = ef_t (t = 0..m);  efb slice s = ef_{Tdev-1-s}
        eff = np.ascontiguousarray(
            ef[:, : m + 1, :].transpose(2, 1, 0), dtype=np.float32
        ).reshape(N_TAGS, (m + 1) * NB).astype(ml_dtypes.bfloat16)
        efb = np.ascontiguousarray(
            ef[:, :m:-1, :].transpose(2, 1, 0), dtype=np.float32
        ).reshape(N_TAGS, S * NB).astype(ml_dtypes.bfloat16)

        # initial backward state: entry value pre-folded for columns that
        # enter the device backward chain; plain vinit otherwise
        u0 = vinit.copy()
        for k in range(NB):
            if int(lmin[k]) > m + 1:
                u0[:, k] = ef[k, int(lmin[k]) - 1, :] * vinit[:, k]
        u0 = u0.astype(np.float32).astype(ml_dtypes.bfloat16)

        boot = np.concatenate(
            [Ebf, EtBf, u0, eff[:, : c0f * NB], efb[:, : c0b * NB]], axis=1
        )
        in_maps.append({"boot": boot, "eff": eff, "efb": efb})

    key = (tuple(act), m, S)
    if key not in _program_cache:
        _program_cache[key] = _build_program(act, m, S, Tdev)
    nc = _program_cache[key]

    _last_nc, _last_in_maps = nc, in_maps
    res = run_bass_kernel_spmd(nc, in_maps, list(range(NCORES)))
    _last_results = res

    # --- host assembly (float64) ---
    partition = np.zeros(bsz)
    for c in range(NCORES):
        qv = res.results[c]["qv_out"].astype(np.float64)         # [128, 2*NB]
        qf = qv[:, :NB]
        vf = qv[:, NB:]
        for k in range(NB):
            b = idx_all[c, k]
            vk = vf[:, k] if int(lmin[k]) > m + 1 else vinit_all[c][:, k]
            # columns with lmin == 1 never enter the device forward chain
            qk = qf[:, k] if int(lmin[k]) > 1 else ef0_all[c][:, k]
            partition[b] = (
                np.log(vk @ qk) + corr_all[c, k] + vlog_all[c, k]
            )

    # --- gold path score (host, float64) ---
    maskf = mask.astype(np.float64)
    trans_tt = lt64[tags[:, :-1], tags[:, 1:]]
    emis = np.take_along_axis(
        feats64[:, :-1, :], tags[:, :-1, None].astype(np.int64), axis=2
    )[..., 0]
    scores = lt64[ROOT, tags[:, 0]]
    scores = scores + (trans_tt * maskf[:, 1:] + emis * maskf[:, :-1]).sum(axis=1)
    last_idx = (maskf.sum(axis=1) - 1.0).astype(np.int64)
    last_tags = np.take_along_axis(np.asarray(tags, np.int64), last_idx[:, None], axis=1)[:, 0]
    last_input = np.take_along_axis(feats64[:, -1, :], last_tags[:, None], axis=1)[:, 0]
    scores = scores + lt64[last_tags, END] + last_input * maskf[:, -1]

    return np.asarray((partition - scores).mean(), dtype=np.float32)
